# revision 15
# baseline (speedup 1.0000x reference)
"""MDCA loss kernel for Trainium2, 8 NeuronCores, data-parallel over batch.

reference:
    counts[c]   = histogram(target) ; avg_count = counts/B
    avg_conf    = mean(logits, axis=1)            # [E, C]
    loss[e]     = mean_c |avg_conf[e,c] - avg_count[c]|

Strategy per core (batch shard of 1024 rows, partition p holds rows 8p..8p+7):
  - the 16.4 MB logits shard streams over the TWO HWDGE queues only
    (sync + scalar).  SWDGE (gpsimd) is avoided for bulk data: its
    SBUF-resident descriptor rings sit on AXI ports shared with SDMA
    engines 7/15, which then lag and straggle the stream end by ~4 us.
    16 KB lines (4 contiguous rows/partition) hit the engines' best
    per-descriptor rate; the HBM QoS governor caps the aggregate at
    ~330 GB/s after ~15 us — that is the wire floor.  All dma_starts are
    emitted first so both rings stay full end-to-end.
  - SDMA round-robin is packet-fair, so ring position == arrival order:
    scalar's ring (40 packet-rounds) empties before sync's (56 rounds),
    and sync ends with four 0.5 MB single-row closers DMA'd straight
    into f32r tiles (bitcast).  Closers need NO DVE fold — just two
    ~390 ns PE matmuls each — so every fold of the 2 MB chunks happens
    while the closers drain, and the post-stream tail is ~1 us of PE.
  - target arrives as [128, 128] f32 (each row-target replicated 16x ->
    512 B lines, no read-modify-write descriptors) at the head of
    scalar's ring; iota runs on the otherwise-idle gpsimd immediately,
    so the histogram phase finishes ~20 us before the first conf matmul
  - ONE PSUM accumulation chain per 500-column half: opened by the
    histogram matmuls ([128,4] -1 weights broadcast -count to the 4 exit
    rows), continued by f32r conf matmuls ([128,4] selector with ones in
    column e folds rows+partitions), closed by the last closer:
    psum[e,c] = sum_conf[e,c] - count[c] with no combine step
  - DVE folds 4-row chunks pairwise into [128, 1000] f32r tiles
  - tail: 2 matmuls + 2 tiny PSUM->SBUF copies + two parallel 8 KB
    stores, one per HWDGE ring
  - host sums the 8 per-core partials and takes |.|-mean / (B*C) -> loss[4]
    (an on-device AllReduce costs ~35 us for 16 KB; host finish wins)
"""

import os
import sys

for _p in ("/opt/trn_rl_repo", "/root/.axon_site/_ro/trn_rl_repo"):
    if os.path.isdir(_p) and _p not in sys.path:
        sys.path.insert(0, _p)

import numpy as np

import concourse.bass as bass
import concourse.bacc as bacc
import concourse.tile as tile
import concourse.mybir as mybir
from concourse.bass_utils import run_bass_kernel_spmd

E, B, C = 4, 8192, 1000
N_CORES = 8
BS = B // N_CORES          # 1024 batch rows per core
GP = 8                     # rows folded per partition (BS = 128 * GP)
CH = C // 2                # 500, C half per PSUM bank
REP = 16                   # target replication -> 512 B DMA lines
F32 = mybir.dt.float32
F32R = mybir.dt.float32r
BF16 = mybir.dt.bfloat16


def build_nc():
    nc = bacc.Bacc(
        "TRN2",
        target_bir_lowering=False,
        debug=False,
        enable_asserts=False,
        num_devices=N_CORES,
    )

    logits = nc.dram_tensor("logits", [E, BS, C], F32, kind="ExternalInput")
    target = nc.dram_tensor("target_f", [128, GP * REP], F32,
                            kind="ExternalInput")
    part_out = nc.dram_tensor("part", [E, C], F32, kind="ExternalOutput")

    # per-exit view: partition p holds rows 8p..8p+7
    src = [logits[e].rearrange("(p i) c -> p i c", i=GP) for e in range(E)]

    with tile.TileContext(nc) as tc:
        with (
            tc.tile_pool(name="const", bufs=1) as const,
            tc.tile_pool(name="ld2", bufs=7) as ld2,
            tc.tile_pool(name="ldz", bufs=4) as ldz,
            tc.tile_pool(name="fold", bufs=4) as foldp,
            tc.tile_pool(name="work", bufs=3) as work,
            tc.tile_pool(name="psum", bufs=1, space=bass.MemorySpace.PSUM) as psum,
        ):
            # ---- phase 1: every load DMA first, so both HWDGE rings
            # fill immediately and drain back-to-back
            def ld_dma(eng, pool, tag, e, r0, r1, dt=F32):
                rows = r1 - r0
                t = pool.tile([128, rows * C], dt, tag=tag,
                              name=f"{tag}_e{e}r{r0}")
                in_ = src[e][:, r0:r1, :]
                if dt is F32R:
                    in_ = in_.bitcast(F32R)
                eng.dma_start(
                    out=t.rearrange("p (i c) -> p i c", i=rows), in_=in_
                )
                return t

            # sync ring: 3x 2MB + 4 single-row closers (56 packet-rounds)
            tA_s = ld_dma(nc.sync, ld2, "ld2", 0, 0, 4)
            tB_s = ld_dma(nc.sync, ld2, "ld2", 1, 4, 8)
            tD_s = ld_dma(nc.sync, ld2, "ld2", 3, 0, 4)
            tZ = [ld_dma(nc.sync, ldz, "ldz", 3, 4 + i, 5 + i, F32R)
                  for i in range(4)]

            # scalar ring: target + 4x 2MB (40 packet-rounds)
            tgt_sb = const.tile([128, GP * REP], F32, tag="tgt")
            nc.scalar.dma_start(out=tgt_sb[:], in_=target[:])
            tA_c = ld_dma(nc.scalar, ld2, "ld2", 1, 0, 4)
            tB_c = ld_dma(nc.scalar, ld2, "ld2", 2, 4, 8)
            tC_c = ld_dma(nc.scalar, ld2, "ld2", 0, 4, 8)
            tE_c = ld_dma(nc.scalar, ld2, "ld2", 2, 0, 4)

            # ---- phase 2: constants (DVE + idle gpsimd)
            sels_f = const.tile([128, 4 * E], F32, tag="sels_f")
            nc.vector.memset(sels_f[:], 0.0)
            for e in range(E):
                nc.vector.memset(sels_f[:, 4 * e + e : 4 * e + e + 1], 1.0)
            sels = const.tile([128, 4 * E], F32R, tag="sels")
            nc.vector.tensor_copy(sels[:], sels_f[:])
            neg1 = const.tile([128, E], BF16, tag="neg1")
            nc.vector.memset(neg1[:], -1.0)
            dummy_rhs = const.tile([128, CH], BF16, tag="dummy")
            nc.vector.memset(dummy_rhs[:], 1.0)
            iota_f = const.tile([128, C], F32, tag="iota")
            nc.gpsimd.iota(
                iota_f[:],
                pattern=[[1, C]],
                base=0,
                channel_multiplier=0,
                allow_small_or_imprecise_dtypes=True,
            )

            # one merged PSUM accumulation chain per column half
            pbank = [
                psum.tile([E, CH], F32, tag=f"pc{h}", name=f"pc{h}")
                for h in range(2)
            ]

            # ---- phase 3: histogram one-hots + chain-opening matmuls
            for i in range(GP):
                onehot = work.tile([128, C], BF16, tag="onehot")
                nc.vector.tensor_scalar(
                    onehot[:],
                    iota_f[:],
                    tgt_sb[:, REP * i : REP * i + 1],
                    None,
                    mybir.AluOpType.is_equal,
                )
                for h in range(2):
                    nc.tensor.matmul(
                        pbank[h][:],
                        neg1[:],
                        onehot[:, h * CH : (h + 1) * CH],
                        start=(i == 0),
                        stop=False,
                    )

            # ---- phase 4: folds + conf matmuls in expected arrival order
            def mm(f, e, stop=False):
                for h in range(2):
                    nc.tensor.matmul(
                        pbank[h][:],
                        sels[:, 4 * e : 4 * e + 4],
                        f[:, h * CH : (h + 1) * CH],
                        start=False,
                        stop=stop,
                    )

            def fold4(t, e):
                for g in range(2):
                    f = foldp.tile([128, C], F32R, tag="fold",
                                   name=f"f_e{e}g{g}")
                    nc.vector.tensor_add(
                        f[:], t[:, 2 * g * C : (2 * g + 1) * C],
                        t[:, (2 * g + 1) * C : (2 * g + 2) * C],
                    )
                    mm(f, e)

            # ring-position arrival order:
            #   A_s@8, A_c@16, B_s@16, B_c@24, D_s@24, C_c@32, Z1@32,
            #   Z2@40, E_c@40, Z3@48, Z4@56  (packet-rounds)
            fold4(tA_s, 0)
            fold4(tA_c, 1)
            fold4(tB_s, 1)
            fold4(tB_c, 2)
            fold4(tD_s, 3)
            fold4(tC_c, 0)
            mm(tZ[0], 3)
            mm(tZ[1], 3)

            # dummy matmuls into a scratch PSUM bank: keep the PE clock
            # ramped through its idle window so the tail matmuls run at
            # ~390 ns instead of ~620 ns (cold pstate)
            scratch = psum.tile([E, CH], F32, tag="scratch")
            for k in range(5):
                nc.tensor.matmul(
                    scratch[:],
                    neg1[:],
                    dummy_rhs[:],
                    start=(k == 0),
                    stop=(k == 4),
                )

            # last fold chunk: run its two row-group folds on DVE and the
            # otherwise-idle gpsimd in parallel (halves the tail fold)
            fE0 = foldp.tile([128, C], F32R, tag="fold", name="fE0")
            nc.vector.tensor_add(fE0[:], tE_c[:, :C], tE_c[:, C : 2 * C])
            fE1 = foldp.tile([128, C], F32R, tag="fold", name="fE1")
            nc.gpsimd.tensor_add(fE1[:], tE_c[:, 2 * C : 3 * C],
                                 tE_c[:, 3 * C :])
            mm(fE0, 2)
            mm(fE1, 2)
            mm(tZ[2], 3)
            mm(tZ[3], 3, stop=True)

            # ---- phase 5: PSUM->SBUF copies + parallel 8 KB stores
            sb0 = work.tile([E, CH], F32, tag="sb0")
            nc.vector.tensor_copy(sb0[:], pbank[0][:])
            nc.sync.dma_start(out=part_out[:, 0:CH], in_=sb0[:])
            sb1 = work.tile([E, CH], F32, tag="sb1")
            nc.vector.tensor_copy(sb1[:], pbank[1][:])
            nc.scalar.dma_start(out=part_out[:, CH:C], in_=sb1[:])

    nc.compile()
    return nc


_NC_CACHE = {}


def _get_nc():
    if "nc" not in _NC_CACHE:
        _NC_CACHE["nc"] = build_nc()
    return _NC_CACHE["nc"]


def make_in_maps(logits: np.ndarray, target: np.ndarray):
    logits = np.ascontiguousarray(logits, dtype=np.float32)
    target = np.asarray(target)
    in_maps = []
    for c in range(N_CORES):
        lg = logits[:, c * BS : (c + 1) * BS, :]
        tg = np.repeat(
            target[c * BS : (c + 1) * BS].astype(np.float32).reshape(128, GP),
            REP, axis=1,
        )
        in_maps.append(
            {"logits": np.ascontiguousarray(lg),
             "target_f": np.ascontiguousarray(tg)}
        )
    return in_maps


def kernel(logits: np.ndarray, target: np.ndarray) -> np.ndarray:
    nc = _get_nc()
    in_maps = make_in_maps(logits, target)
    res = run_bass_kernel_spmd(nc, in_maps, core_ids=list(range(N_CORES)))
    parts = sum(np.asarray(r["part"], dtype=np.float64) for r in res.results)
    return (np.abs(parts).sum(axis=1) / (B * C)).astype(np.float32)


# revision 17
# speedup vs baseline: 1.0320x; 1.0320x over previous
"""MDCA loss kernel for Trainium2, 8 NeuronCores, data-parallel over batch.

reference:
    counts[c]   = histogram(target) ; avg_count = counts/B
    avg_conf    = mean(logits, axis=1)            # [E, C]
    loss[e]     = mean_c |avg_conf[e,c] - avg_count[c]|

Strategy per core (batch shard of 1024 rows, partition p holds rows 8p..8p+7):
  - the 16.4 MB logits shard streams over the TWO HWDGE queues only
    (sync + scalar).  SWDGE (gpsimd) is avoided for bulk data: its
    SBUF-resident descriptor rings sit on AXI ports shared with SDMA
    engines 7/15, which then lag and straggle the stream end by ~4 us,
    and with SWDGE in the mix the HBM QoS governor throttles the
    aggregate to ~330 GB/s.  Pure-HWDGE streaming with 16 KB lines
    (4 contiguous rows/partition, the engines' best per-descriptor
    rate) sustains ~425 GB/s/core for the whole stream.  All dma_starts
    are emitted first so both rings stay full end-to-end.
  - SDMA round-robin is packet-fair, so ring position == arrival order:
    scalar's ring (40 packet-rounds) empties before sync's (56 rounds),
    and sync ends with four 0.5 MB single-row closers DMA'd straight
    into f32r tiles (bitcast).  Closers need NO DVE fold — just two
    ~390 ns PE matmuls each — so every fold of the 2 MB chunks happens
    while the closers drain, and the post-stream tail is ~1 us of PE.
  - target arrives as [128, 128] f32 (each row-target replicated 16x ->
    512 B lines, no read-modify-write descriptors) at the head of
    scalar's ring; iota runs on the otherwise-idle gpsimd immediately,
    so the histogram phase finishes ~20 us before the first conf matmul
  - ONE PSUM accumulation chain per 500-column half: opened by the
    histogram matmuls ([128,4] -1 weights broadcast -count to the 4 exit
    rows), continued by f32r conf matmuls ([128,4] selector with ones in
    column e folds rows+partitions), closed by the last closer:
    psum[e,c] = sum_conf[e,c] - count[c] with no combine step
  - DVE folds 4-row chunks pairwise into [128, 1000] f32r tiles
  - tail: 2 matmuls + 2 tiny PSUM->SBUF copies + two parallel 8 KB
    stores, one per HWDGE ring
  - host sums the 8 per-core partials and takes |.|-mean / (B*C) -> loss[4]
    (an on-device AllReduce costs ~35 us for 16 KB; host finish wins)
"""

import os
import sys

for _p in ("/opt/trn_rl_repo", "/root/.axon_site/_ro/trn_rl_repo"):
    if os.path.isdir(_p) and _p not in sys.path:
        sys.path.insert(0, _p)

import numpy as np

import concourse.bass as bass
import concourse.bacc as bacc
import concourse.tile as tile
import concourse.mybir as mybir
from concourse.bass_utils import run_bass_kernel_spmd

E, B, C = 4, 8192, 1000
N_CORES = 8
BS = B // N_CORES          # 1024 batch rows per core
GP = 8                     # rows folded per partition (BS = 128 * GP)
CH = C // 2                # 500, C half per PSUM bank
REP = 16                   # target replication -> 512 B DMA lines
F32 = mybir.dt.float32
F32R = mybir.dt.float32r
BF16 = mybir.dt.bfloat16


def build_nc():
    nc = bacc.Bacc(
        "TRN2",
        target_bir_lowering=False,
        debug=False,
        enable_asserts=False,
        num_devices=N_CORES,
    )

    logits = nc.dram_tensor("logits", [E, BS, C], F32, kind="ExternalInput")
    target = nc.dram_tensor("target_f", [128, GP * REP], F32,
                            kind="ExternalInput")
    part_out = nc.dram_tensor("part", [E, C], F32, kind="ExternalOutput")

    # per-exit view: partition p holds rows 8p..8p+7
    src = [logits[e].rearrange("(p i) c -> p i c", i=GP) for e in range(E)]

    with tile.TileContext(nc) as tc:
        with (
            tc.tile_pool(name="const", bufs=1) as const,
            tc.tile_pool(name="ld2", bufs=7) as ld2,
            tc.tile_pool(name="ldz", bufs=4) as ldz,
            tc.tile_pool(name="fold", bufs=4) as foldp,
            tc.tile_pool(name="work", bufs=3) as work,
            tc.tile_pool(name="psum", bufs=1, space=bass.MemorySpace.PSUM) as psum,
        ):
            # ---- phase 1: every load DMA first, so both HWDGE rings
            # fill immediately and drain back-to-back
            def ld_dma(eng, pool, tag, e, r0, r1, dt=F32):
                rows = r1 - r0
                t = pool.tile([128, rows * C], dt, tag=tag,
                              name=f"{tag}_e{e}r{r0}")
                in_ = src[e][:, r0:r1, :]
                if dt is F32R:
                    in_ = in_.bitcast(F32R)
                eng.dma_start(
                    out=t.rearrange("p (i c) -> p i c", i=rows), in_=in_
                )
                return t

            # sync ring: 3x 2MB + 4 single-row closers (56 packet-rounds)
            tA_s = ld_dma(nc.sync, ld2, "ld2", 0, 0, 4)
            tB_s = ld_dma(nc.sync, ld2, "ld2", 1, 4, 8)
            tD_s = ld_dma(nc.sync, ld2, "ld2", 3, 0, 4)
            tZ = [ld_dma(nc.sync, ldz, "ldz", 3, 4 + i, 5 + i, F32R)
                  for i in range(4)]

            # scalar ring: target + 4x 2MB (40 packet-rounds)
            tgt_sb = const.tile([128, GP * REP], F32, tag="tgt")
            nc.scalar.dma_start(out=tgt_sb[:], in_=target[:])
            tA_c = ld_dma(nc.scalar, ld2, "ld2", 1, 0, 4)
            tB_c = ld_dma(nc.scalar, ld2, "ld2", 2, 4, 8)
            tC_c = ld_dma(nc.scalar, ld2, "ld2", 0, 4, 8)
            tE_c = ld_dma(nc.scalar, ld2, "ld2", 2, 0, 4)

            # ---- phase 2: constants (DVE + idle gpsimd)
            sels_f = const.tile([128, 4 * E], F32, tag="sels_f")
            nc.vector.memset(sels_f[:], 0.0)
            for e in range(E):
                nc.vector.memset(sels_f[:, 4 * e + e : 4 * e + e + 1], 1.0)
            sels = const.tile([128, 4 * E], F32R, tag="sels")
            nc.vector.tensor_copy(sels[:], sels_f[:])
            neg1 = const.tile([128, E], BF16, tag="neg1")
            nc.vector.memset(neg1[:], -1.0)
            iota_f = const.tile([128, C], F32, tag="iota")
            nc.gpsimd.iota(
                iota_f[:],
                pattern=[[1, C]],
                base=0,
                channel_multiplier=0,
                allow_small_or_imprecise_dtypes=True,
            )

            # one merged PSUM accumulation chain per column half
            pbank = [
                psum.tile([E, CH], F32, tag=f"pc{h}", name=f"pc{h}")
                for h in range(2)
            ]

            # ---- phase 3: histogram one-hots + chain-opening matmuls
            for i in range(GP):
                onehot = work.tile([128, C], BF16, tag="onehot")
                nc.vector.tensor_scalar(
                    onehot[:],
                    iota_f[:],
                    tgt_sb[:, REP * i : REP * i + 1],
                    None,
                    mybir.AluOpType.is_equal,
                )
                for h in range(2):
                    nc.tensor.matmul(
                        pbank[h][:],
                        neg1[:],
                        onehot[:, h * CH : (h + 1) * CH],
                        start=(i == 0),
                        stop=False,
                    )

            # ---- phase 4: folds + conf matmuls in expected arrival order
            def mm(f, e, stop=False):
                for h in range(2):
                    nc.tensor.matmul(
                        pbank[h][:],
                        sels[:, 4 * e : 4 * e + 4],
                        f[:, h * CH : (h + 1) * CH],
                        start=False,
                        stop=stop,
                    )

            def fold4(t, e):
                for g in range(2):
                    f = foldp.tile([128, C], F32R, tag="fold",
                                   name=f"f_e{e}g{g}")
                    nc.vector.tensor_add(
                        f[:], t[:, 2 * g * C : (2 * g + 1) * C],
                        t[:, (2 * g + 1) * C : (2 * g + 2) * C],
                    )
                    mm(f, e)

            # ring-position arrival order:
            #   A_s@8, A_c@16, B_s@16, B_c@24, D_s@24, C_c@32, Z1@32,
            #   Z2@40, E_c@40, Z3@48, Z4@56  (packet-rounds)
            fold4(tA_s, 0)
            fold4(tA_c, 1)
            fold4(tB_s, 1)
            fold4(tB_c, 2)
            fold4(tD_s, 3)
            fold4(tC_c, 0)
            mm(tZ[0], 3)
            mm(tZ[1], 3)
            fold4(tE_c, 2)
            mm(tZ[2], 3)
            mm(tZ[3], 3, stop=True)

            # ---- phase 5: PSUM->SBUF copies + parallel 8 KB stores
            sb0 = work.tile([E, CH], F32, tag="sb0")
            nc.vector.tensor_copy(sb0[:], pbank[0][:])
            nc.sync.dma_start(out=part_out[:, 0:CH], in_=sb0[:])
            sb1 = work.tile([E, CH], F32, tag="sb1")
            nc.vector.tensor_copy(sb1[:], pbank[1][:])
            nc.scalar.dma_start(out=part_out[:, CH:C], in_=sb1[:])

    nc.compile()
    return nc


_NC_CACHE = {}


def _get_nc():
    if "nc" not in _NC_CACHE:
        _NC_CACHE["nc"] = build_nc()
    return _NC_CACHE["nc"]


def make_in_maps(logits: np.ndarray, target: np.ndarray):
    logits = np.ascontiguousarray(logits, dtype=np.float32)
    target = np.asarray(target)
    in_maps = []
    for c in range(N_CORES):
        lg = logits[:, c * BS : (c + 1) * BS, :]
        tg = np.repeat(
            target[c * BS : (c + 1) * BS].astype(np.float32).reshape(128, GP),
            REP, axis=1,
        )
        in_maps.append(
            {"logits": np.ascontiguousarray(lg),
             "target_f": np.ascontiguousarray(tg)}
        )
    return in_maps


def kernel(logits: np.ndarray, target: np.ndarray) -> np.ndarray:
    nc = _get_nc()
    in_maps = make_in_maps(logits, target)
    res = run_bass_kernel_spmd(nc, in_maps, core_ids=list(range(N_CORES)))
    parts = sum(np.asarray(r["part"], dtype=np.float64) for r in res.results)
    return (np.abs(parts).sum(axis=1) / (B * C)).astype(np.float32)
